# revision 38
# baseline (speedup 1.0000x reference)
"""Trainium2 Bass kernel for triple-head Bahdanau attention (nn_Attention_48258252537865).

Reference computation (S=8192, H2=1024, A=2048, E=768):
  for each head t in {pos, cardinal, headline}:
      u_t = sentence @ W_sent_t + b_sent_t + (ctx_t @ W_ctx_t + b_ctx_t)   [1,S,A]
      e_t = tanh(u_t) @ v_t + bv_t                                          [1,S]
      w_t = softmax(mask(e_t))
  fused = (w_p + w_c + w_h) / 3
  out = fused @ sentence                                                    [1,H2]

Strategy: sequence-parallel over 8 NeuronCores; each core handles S/8 rows and
emits per-head (Z, N) partial softmax sums which the host combines exactly.

Versus the f32r baseline (246 us -> ~196 us):
  - U_FP8_PAIRS k-tile PAIRS of the u contraction run as fp8e4 DoubleRow
    matmuls (2 k-tiles per PE pass); the remaining k-tiles run in bf16.
    pairs=2 keeps the end-to-end rel err ~1.4-1.7e-2 under every reasonable
    norm (max/max, L2/L2, mean) vs the 2e-2 gate; pairs=4 would be ~25%
    faster still but its L2/L2 error (2.03e-2) sits over the gate.
  - all other matmul operands are bf16: halves HBM traffic and LDWEIGHTS
    time versus f32r at ~2e-3 rel err.
  - the softmax max-subtraction is dropped: |e| <= sum|v| ~ 36 so exp(e)
    fits fp32 easily, removing the serial two-pass max and the per-core
    log-sum-exp combine (the host just sums Z and N).
  - the u accumulation carries a uniform x16 scale (W*16 in bf16, or W*8
    and x*2 for the fp8 pairs) undone by the tanh activation's scale=1/16.
  - head: the first W tile is DMA-split per k-pair and interleaved with the
    sentT chunks in exact consumption order; tail: the ScalarE Exp table is
    preloaded while the PE drains the last score matmuls.
"""

import numpy as np
from contextlib import ExitStack

S = 8192
H2 = 1024
A = 2048
NCORES = 8
NEG = -1.0e30

# Number of u-contraction k-tile pairs (of KT//2 = 4) computed in fp8e4 with
# DoubleRow (2x PE throughput); the remaining k-tiles run in bf16.
U_FP8_PAIRS = 2

_cache = {}
LAST_RESULTS = None  # BassKernelResults of the most recent device run


def _build(S_local, pairs):
    import concourse.bacc as bacc
    import concourse.tile as tile
    from concourse import mybir

    F32 = mybir.dt.float32
    BF16 = mybir.dt.bfloat16
    FP8 = mybir.dt.float8e4
    DR = mybir.MatmulPerfMode.DoubleRow
    TANH = mybir.ActivationFunctionType.Tanh
    EXP = mybir.ActivationFunctionType.Exp

    KT = H2 // 128                      # contraction k-tiles for u
    NK8 = 2 * pairs                     # k-tiles in fp8
    NKB = KT - NK8                      # k-tiles in bf16
    NJ = A // 128                       # a-tiles per head
    ST = S_local // 128                 # s-tiles (transpose/numerator)
    SC = [(c, min(512, S_local - c)) for c in range(0, S_local, 512)]

    nc = bacc.Bacc("TRN2", target_bir_lowering=False, debug=False,
                   num_devices=NCORES)

    if NK8:
        sT8_d = nc.dram_tensor("sT8", [NK8 * 128, S_local], FP8,
                               kind="ExternalInput")
        W8_d = nc.dram_tensor("W8", [3, NJ, 128, NK8 * 128], FP8,
                              kind="ExternalInput")
    if NKB:
        sTb_d = nc.dram_tensor("sTb", [NKB * 128, S_local], BF16,
                               kind="ExternalInput")
        Wb_d = nc.dram_tensor("Wb", [3, NJ, 128, NKB * 128], BF16,
                              kind="ExternalInput")
    sent_d = nc.dram_tensor("sent", [S_local, H2], BF16, kind="ExternalInput")
    Vt_d = nc.dram_tensor("Vt", [128, 3 * NJ * 4], BF16, kind="ExternalInput")
    Bt_d = nc.dram_tensor("Bt", [128, 3 * NJ], F32, kind="ExternalInput")
    mask_d = nc.dram_tensor("mask1", [1, S_local], BF16, kind="ExternalInput")
    ones_d = nc.dram_tensor("ones3", [1, 3], BF16, kind="ExternalInput")
    id3_d = nc.dram_tensor("id3", [3, 3], BF16, kind="ExternalInput")

    Ncore_d = nc.dram_tensor("Ncore", [3, H2], F32, kind="ExternalOutput")
    stats_d = nc.dram_tensor("stats", [3, 1], F32, kind="ExternalOutput")

    with tile.TileContext(nc) as tc, ExitStack() as ctx:
        const = ctx.enter_context(tc.tile_pool(name="const", bufs=1))
        wpool = ctx.enter_context(tc.tile_pool(name="w", bufs=10))
        thpool = ctx.enter_context(tc.tile_pool(name="th", bufs=6))
        # phase-1 PSUM pools (all 8 banks); closed before the epilogue pools
        # open so the banks can be reused
        ph1 = ExitStack()
        upool = ph1.enter_context(tc.tile_pool(name="u", bufs=6, space="PSUM"))
        epool = ph1.enter_context(tc.tile_pool(name="e", bufs=1, space="PSUM"))

        # ---- sync HWDGE ring: the first weight tiles first (they gate the
        # first matmul), interleaved with sentT chunk-0 per-k 2D transfers ----
        Wt_sb = {}

        def _wdma(t, j, ring=None):
            ring = ring or nc.sync
            tiles = []
            if NK8:
                w8 = wpool.tile([128, NK8 * 128], FP8, tag="w8")
                ring.dma_start(w8[:], W8_d.ap()[t, j])
                tiles.append(w8)
            else:
                tiles.append(None)
            if NKB:
                wb = wpool.tile([128, NKB * 128], BF16, tag="wb")
                ring.dma_start(wb[:], Wb_d.ap()[t, j])
                tiles.append(wb)
            else:
                tiles.append(None)
            Wt_sb[(t, j)] = tiles

        if NK8:
            sT8_sb = const.tile([128, NK8 * S_local], FP8, tag="sT8")
        if NKB:
            sTb_sb = const.tile([128, NKB * S_local], BF16, tag="sTb")

        def _sdma(ring, k, c, n):
            # sentT k-tile k in 256-col pieces (more DMA-engine parallelism
            # during the bandwidth-bound head phase)
            for cc in range(c, c + n, 256):
                nn = min(256, c + n - cc)
                if k < NK8:
                    ring.dma_start(
                        sT8_sb[:, k * S_local + cc: k * S_local + cc + nn],
                        sT8_d.ap()[k * 128:(k + 1) * 128, cc:cc + nn])
                else:
                    kb = k - NK8
                    ring.dma_start(
                        sTb_sb[:, kb * S_local + cc: kb * S_local + cc + nn],
                        sTb_d.ap()[kb * 128:(kb + 1) * 128, cc:cc + nn])

        c0, n0 = SC[0]
        if NK8:
            # split the gating first W tile per k-pair so the first DoubleRow
            # matmul (which needs only kp=0's 32KB + sentT k0/k1) starts early
            w8_00 = wpool.tile([128, NK8 * 128], FP8, tag="w8")
            for kp in range(pairs):
                # sync ring streams in exact consumption order of (0,0)'s
                # chunk-0 matmuls; the chunk-1 halves ride the idle vector
                # ring so they don't queue ahead of the (0,1..3) weights
                nc.sync.dma_start(w8_00[:, kp * 256:(kp + 1) * 256],
                                  W8_d.ap()[0, 0, :, kp * 256:(kp + 1) * 256])
                for (c, n) in SC:
                    _sdma(nc.sync, 2 * kp, c, n)
                    _sdma(nc.sync, 2 * kp + 1, c, n)
            wb_00 = None
            if NKB:
                # split (0,0)'s bf16 weights per k-tile too, in consumption
                # order with their sentT chunk-0 tiles
                wb_00 = wpool.tile([128, NKB * 128], BF16, tag="wb")
                for k in range(NK8, KT):
                    kb = k - NK8
                    nc.sync.dma_start(wb_00[:, kb * 128:(kb + 1) * 128],
                                      Wb_d.ap()[0, 0, :, kb * 128:(kb + 1) * 128])
                    _sdma(nc.sync, k, c0, n0)
                for (c, n) in SC[1:]:
                    for k in range(NK8, KT):
                        _sdma(nc.gpsimd, k, c, n)
            Wt_sb[(0, 0)] = [w8_00, wb_00]
            _wdma(0, 1)
            _wdma(0, 2)
            _wdma(0, 3)
        else:
            _wdma(0, 0)
            for k in range(KT // 2):
                _sdma(nc.sync, k, c0, n0)
            _wdma(0, 1)
            for k in range(KT // 2, KT):
                _sdma(nc.sync, k, c0, n0)
            for (c, n) in SC[1:]:
                for k in range(KT):
                    _sdma(nc.gpsimd, k, c, n)
            _wdma(0, 2)
            _wdma(0, 3)

        # ---- consts on the scalar HWDGE ring (separate FIFO) ----
        Vt_sb = const.tile([128, 3 * NJ * 4], BF16, tag="vt")
        Bt_sb = const.tile([128, 3 * NJ], F32, tag="bt")
        mask_sb = const.tile([1, S_local], BF16, tag="mask")
        ones_sb = const.tile([1, 3], BF16, tag="ones")
        id3_sb = const.tile([3, 3], BF16, tag="id3")
        nc.scalar.dma_start(Bt_sb[:], Bt_d.ap()[:])
        nc.scalar.dma_start(Vt_sb[:], Vt_d.ap()[:])
        nc.scalar.dma_start(mask_sb[:], mask_d.ap()[:])
        nc.scalar.dma_start(ones_sb[:], ones_d.ap()[:])
        nc.scalar.dma_start(id3_sb[:], id3_d.ap()[:])

        # ---- the big numerator operand rides the SWDGE ring since it isn't
        # needed until the epilogue ----
        sent_sb = const.tile([128, ST * H2], BF16, tag="sent")
        nc.gpsimd.dma_start(sent_sb[:].rearrange("p (k h) -> p k h", k=ST),
                            sent_d.ap().rearrange("(k p) h -> p k h", p=128))

        # ---- score accumulator [3, S_local]: head t on partition t ----
        e3_ps = epool.tile([3, S_local], F32, tag="e")

        # ---- three heads: u -> tanh -> scores ----
        pend = None  # tanh tile of the previous (t, j), awaiting score matmuls
        for t in range(3):
            for j in range(NJ):
                w8, wb = Wt_sb.pop((t, j), (None, None))
                if NK8 and w8 is None:
                    w8 = wpool.tile([128, NK8 * 128], FP8, tag="w8")
                    nc.sync.dma_start(w8[:], W8_d.ap()[t, j])
                if NKB and wb is None:
                    wb = wpool.tile([128, NKB * 128], BF16, tag="wb")
                    nc.sync.dma_start(wb[:], Wb_d.ap()[t, j])
                th = thpool.tile([128, S_local], BF16, tag="th")
                if NK8:
                    w8v = w8[:].rearrange("p (kp i m) -> p kp i m",
                                          kp=pairs, i=2)
                    s8v = sT8_sb[:].rearrange("p (k s) -> p k s", k=NK8)
                # per-chunk u tiles + tanh: the first tanh fires a chunk
                # earlier and PSUM recycles at chunk granularity, halving
                # the u-pool backpressure window
                for ci, (c, n) in enumerate(SC):
                    u_psc = upool.tile([128, 512], F32, tag="u")
                    for kp in range(pairs):
                        nc.tensor.matmul(
                            u_psc[:, 0:n],
                            w8v[:, kp],
                            s8v[:, 2 * kp:2 * kp + 2, c:c + n],
                            start=(kp == 0), stop=(kp == pairs - 1 and NKB == 0),
                            perf_mode=DR)
                    for kb in range(NKB):
                        nc.tensor.matmul(
                            u_psc[:, 0:n],
                            wb[:, kb * 128:(kb + 1) * 128],
                            sTb_sb[:, kb * S_local + c: kb * S_local + c + n],
                            start=(kb == 0 and pairs == 0),
                            stop=(kb == NKB - 1))
                    if ci == len(SC) - 1 and pend is not None:
                        pth, pt, pj = pend
                        for (cs, ns) in SC:
                            nc.tensor.matmul(
                                e3_ps[0:3, cs:cs + ns],
                                Vt_sb[:, 4 * (pj * 3 + pt): 4 * (pj * 3 + pt) + 3],
                                pth[:, cs:cs + ns],
                                start=False, stop=(pt == 2 and pj == NJ - 1))
                    nc.scalar.activation(th[:, c:c + n], u_psc[:, 0:n], TANH,
                                         scale=1.0 / 16.0,
                                         bias=Bt_sb[:, j * 3 + t: j * 3 + t + 1])
                pend = (th, t, j)
                if t == 0 and j == 0:
                    # additive key mask enters the score accumulator via a
                    # K=1 ones-matmul; emitted here (after the first u-group)
                    # so it doesn't head the PE queue at startup, but still
                    # precedes every score matmul
                    for (c, n) in SC:
                        nc.tensor.matmul(e3_ps[0:3, c:c + n], ones_sb[:],
                                         mask_sb[0:1, c:c + n],
                                         start=True, stop=False)
        # preload the Exp activation table while the PE finishes the last
        # score matmuls (the table swap costs ~1.7us on the ScalarE and would
        # otherwise land on the serial epilogue path)
        expwarm = const.tile([1, 3], F32, tag="expwarm")
        nc.scalar.activation(expwarm[:], ones_sb[:], EXP)

        pth, pt, pj = pend
        for (c, n) in SC:
            nc.tensor.matmul(e3_ps[0:3, c:c + n],
                             Vt_sb[:, 4 * (pj * 3 + pt): 4 * (pj * 3 + pt) + 3],
                             pth[:, c:c + n], start=False,
                             stop=True)

        # ---- no-max softmax: exp straight off PSUM (|e| <= ~36 so exp fits
        # fp32 with room), accumulating Z along the way ----
        e3x_sb = const.tile([3, S_local], BF16, tag="e3x")
        Z3 = const.tile([3, 1], F32, tag="z3")
        SCE = [(c, min(256, S_local - c)) for c in range(0, S_local, 256)]
        zpart = const.tile([3, len(SCE)], F32, tag="zpart")
        for ci, (c, n) in enumerate(SCE):
            nc.scalar.activation(e3x_sb[0:3, c:c + n], e3_ps[0:3, c:c + n], EXP,
                                 accum_out=zpart[:, ci:ci + 1])
        if len(SCE) > 1:
            nc.vector.reduce_sum(Z3[:, 0:1], zpart[:], axis=mybir.AxisListType.X)
        else:
            nc.vector.tensor_copy(Z3[:, 0:1], zpart[:, 0:1])
        stats_sb = const.tile([3, 1], F32, tag="stats")
        nc.vector.tensor_copy(stats_sb[:, 0:1], Z3[:, 0:1])
        nc.scalar.dma_start(stats_d.ap()[:], stats_sb[:])

        ph1.close()  # free u/e PSUM banks for the epilogue pools

        # ---- fused epilogue: per s-tile, transpose exp-scores to [s, 3]
        # and immediately accumulate both H2 halves of the numerator
        # N[t, :] = sum_s exp_scores[t, s] * sent[s, :] ----
        trpool = ctx.enter_context(tc.tile_pool(name="tr", bufs=2, space="PSUM"))
        npool = ctx.enter_context(tc.tile_pool(name="n", bufs=2, space="PSUM"))
        eT_sb = const.tile([128, 4 * ST], BF16, tag="eT")
        n_ps = []
        for _hi in range(H2 // 512):
            n_ps_hi = npool.tile([3, 512], F32, tag="n")
            n_ps.append(n_ps_hi)
        for k in range(ST):
            tr_ps = trpool.tile([128, 3], BF16, tag="tr")
            nc.tensor.transpose(tr_ps[:], e3x_sb[0:3, k * 128:(k + 1) * 128],
                                id3_sb[:])
            nc.vector.tensor_copy(eT_sb[:, 4 * k:4 * k + 3], tr_ps[:])
            for hi, hc in enumerate(range(0, H2, 512)):
                nc.tensor.matmul(n_ps[hi][0:3, :],
                                 eT_sb[:, 4 * k:4 * k + 3],
                                 sent_sb[:, k * H2 + hc: k * H2 + hc + 512],
                                 start=(k == 0), stop=(k == ST - 1))
        n_sb = const.tile([3, H2], F32, tag="nsb")
        for hi, hc in enumerate(range(0, H2, 512)):
            nc.vector.tensor_copy(n_sb[:, hc:hc + 512], n_ps[hi][:])
            nc.sync.dma_start(Ncore_d.ap()[:, hc:hc + 512], n_sb[:, hc:hc + 512])

    nc.compile()
    return nc


def kernel(**inputs):
    global LAST_RESULTS
    import ml_dtypes
    from concourse import bass_utils

    E4 = ml_dtypes.float8_e4m3
    BF = ml_dtypes.bfloat16

    sentence = np.ascontiguousarray(
        np.asarray(inputs["sentence"], dtype=np.float32)[0])      # [S, H2]
    length = int(np.asarray(inputs["length"]).reshape(-1)[0])
    if length <= 0:
        return np.zeros((1, H2), dtype=np.float32)
    length = min(length, S)

    ctxs = [inputs["pos_embedding"], inputs["cardinal_phrase_embedding"],
            inputs["headline_embedding"]]
    tags = ["p", "c", "h"]

    # host-side prep: fold ctx projection + b_sent into a single bias [3, A]
    bias_all = np.empty((3, A), dtype=np.float32)
    W_all = np.empty((3, H2, A), dtype=np.float32)
    v_all = np.empty((3, A), dtype=np.float32)
    for i, tg in enumerate(tags):
        ctx = np.asarray(ctxs[i], dtype=np.float32)[0]            # [E]
        bias_all[i] = (np.asarray(inputs[f"b_sent_{tg}"], dtype=np.float32)
                       + ctx @ np.asarray(inputs[f"W_ctx_{tg}"], dtype=np.float32)
                       + np.asarray(inputs[f"b_ctx_{tg}"], dtype=np.float32))
        W_all[i] = np.asarray(inputs[f"W_sent_{tg}"], dtype=np.float32)
        v_all[i] = np.asarray(inputs[f"v_{tg}"], dtype=np.float32)

    pairs = U_FP8_PAIRS
    NK8 = 2 * pairs
    KT = H2 // 128
    NKB = KT - NK8
    S_local = max(128, -(-length // (NCORES * 128)) * 128)        # ceil, 128-aligned
    nc = _cache.get((S_local, pairs))
    if nc is None:
        nc = _build(S_local, pairs)
        _cache[(S_local, pairs)] = nc

    NJ = A // 128
    # W tiles, k-tile major per (t, j):  [3, NJ, 128, KT, 128] with the
    # partition dim holding the low 7 bits of the contraction index
    Wt = (W_all.reshape(3, KT, 128, NJ, 128)
               .transpose(0, 3, 2, 1, 4))                         # [3,NJ,128,KT,128]
    if NK8:
        # fp8 pairs carry W*8 (and x*2) for a uniform x16 PSUM scale
        W8 = np.ascontiguousarray(
            np.clip(Wt[:, :, :, :NK8] * 8.0, -240, 240)).astype(E4)
        W8 = np.ascontiguousarray(W8.reshape(3, NJ, 128, NK8 * 128))
    if NKB:
        Wb = np.ascontiguousarray(Wt[:, :, :, NK8:] * 16.0).astype(BF)
        Wb = np.ascontiguousarray(Wb.reshape(3, NJ, 128, NKB * 128))

    # [128, (j t) * 3]: head t's v-tile in column t of its [128, 3] block
    vt_cols = v_all.T.reshape(NJ, 128, 3).transpose(1, 0, 2)      # [128, NJ, 3]
    Vt = np.zeros((128, NJ, 3, 4), dtype=np.float32)
    for t in range(3):
        Vt[:, :, t, t] = vt_cols[:, :, t]
    Vt = np.ascontiguousarray(Vt.reshape(128, 3 * NJ * 4)).astype(BF)
    Bt = np.ascontiguousarray(
        bias_all.T.reshape(NJ, 128, 3).transpose(1, 0, 2).reshape(128, 3 * NJ))
    id3 = np.eye(3, dtype=np.float32).astype(BF)
    ones3 = np.ones((1, 3), dtype=np.float32).astype(BF)

    in_maps = []
    for c in range(NCORES):
        s0 = c * S_local
        sl = sentence[s0:s0 + S_local]
        if sl.shape[0] < S_local:                                  # pad tail core
            sl = np.concatenate(
                [sl, np.zeros((S_local - sl.shape[0], H2), np.float32)], axis=0)
        mask1 = np.where((s0 + np.arange(S_local))[None, :] < length,
                         0.0, NEG).astype(np.float32).astype(BF)
        slT = sl.T                                                 # [H2, S_local]
        im = dict(Vt=Vt, Bt=Bt, mask1=mask1, ones3=ones3, id3=id3,
                  sent=np.ascontiguousarray(sl).astype(BF))
        if NK8:
            im["sT8"] = np.ascontiguousarray(
                np.clip(slT[:NK8 * 128] * 2.0, -240, 240)).astype(E4)
            im["W8"] = W8
        if NKB:
            im["sTb"] = np.ascontiguousarray(slT[NK8 * 128:]).astype(BF)
            im["Wb"] = Wb
        in_maps.append(im)

    res = bass_utils.run_bass_kernel_spmd(nc, in_maps,
                                          core_ids=list(range(NCORES)))
    LAST_RESULTS = res

    # ---- exact cross-core softmax combine: plain sums (no max shift) ----
    Z = np.zeros(3, dtype=np.float64)
    N = np.zeros((3, H2), dtype=np.float64)
    for c in range(NCORES):
        Z += res.results[c]["stats"][:, 0].astype(np.float64)
        N += res.results[c]["Ncore"].astype(np.float64)
    out = (N / Z[:, None]).mean(axis=0)
    return out[None, :].astype(np.float32)


# revision 39
# speedup vs baseline: 1.0512x; 1.0512x over previous
"""Trainium2 Bass kernel for triple-head Bahdanau attention (nn_Attention_48258252537865).

Reference computation (S=8192, H2=1024, A=2048, E=768):
  for each head t in {pos, cardinal, headline}:
      u_t = sentence @ W_sent_t + b_sent_t + (ctx_t @ W_ctx_t + b_ctx_t)   [1,S,A]
      e_t = tanh(u_t) @ v_t + bv_t                                          [1,S]
      w_t = softmax(mask(e_t))
  fused = (w_p + w_c + w_h) / 3
  out = fused @ sentence                                                    [1,H2]

Strategy: sequence-parallel over 8 NeuronCores; each core handles S/8 rows and
emits per-head (Z, N) partial softmax sums which the host combines exactly.

Versus the f32r baseline (246 us -> ~196 us):
  - U_FP8_PAIRS k-tile PAIRS of the u contraction run as fp8e4 DoubleRow
    matmuls (2 k-tiles per PE pass); the remaining k-tiles run in bf16.
    pairs=2 keeps the end-to-end rel err ~1.4-1.7e-2 under every reasonable
    norm (max/max, L2/L2, mean) vs the 2e-2 gate; pairs=4 would be ~25%
    faster still but its L2/L2 error (2.03e-2) sits over the gate.
  - all other matmul operands are bf16: halves HBM traffic and LDWEIGHTS
    time versus f32r at ~2e-3 rel err.
  - the softmax max-subtraction is dropped: |e| <= sum|v| ~ 36 so exp(e)
    fits fp32 easily, removing the serial two-pass max and the per-core
    log-sum-exp combine (the host just sums Z and N).
  - the u accumulation carries a uniform x16 scale (W*16 in bf16, or W*8
    and x*2 for the fp8 pairs) undone by the tanh activation's scale=1/16.
  - head: the first W tile is DMA-split per k-pair and interleaved with the
    sentT chunks in exact consumption order; tail: the ScalarE Exp table is
    preloaded while the PE drains the last score matmuls.
"""

import numpy as np
from contextlib import ExitStack

S = 8192
H2 = 1024
A = 2048
NCORES = 8
NEG = -1.0e30

# Number of u-contraction k-tile pairs (of KT//2 = 4) computed in fp8e4 with
# DoubleRow (2x PE throughput); the remaining k-tiles run in bf16.
U_FP8_PAIRS = 2

_cache = {}
LAST_RESULTS = None  # BassKernelResults of the most recent device run


def _build(S_local, pairs):
    import concourse.bacc as bacc
    import concourse.tile as tile
    from concourse import mybir

    F32 = mybir.dt.float32
    BF16 = mybir.dt.bfloat16
    FP8 = mybir.dt.float8e4
    DR = mybir.MatmulPerfMode.DoubleRow
    TANH = mybir.ActivationFunctionType.Tanh
    EXP = mybir.ActivationFunctionType.Exp

    KT = H2 // 128                      # contraction k-tiles for u
    NK8 = 2 * pairs                     # k-tiles in fp8
    NKB = KT - NK8                      # k-tiles in bf16
    NJ = A // 128                       # a-tiles per head
    ST = S_local // 128                 # s-tiles (transpose/numerator)
    SC = [(c, min(512, S_local - c)) for c in range(0, S_local, 512)]

    nc = bacc.Bacc("TRN2", target_bir_lowering=False, debug=False,
                   num_devices=NCORES)

    if NK8:
        sT8_d = nc.dram_tensor("sT8", [NK8 * 128, S_local], FP8,
                               kind="ExternalInput")
        W8_d = nc.dram_tensor("W8", [3, NJ, 128, NK8 * 128], FP8,
                              kind="ExternalInput")
    if NKB:
        sTb_d = nc.dram_tensor("sTb", [NKB * 128, S_local], BF16,
                               kind="ExternalInput")
        Wb_d = nc.dram_tensor("Wb", [3, NJ, 128, NKB * 128], BF16,
                              kind="ExternalInput")
    sent_d = nc.dram_tensor("sent", [S_local, H2], BF16, kind="ExternalInput")
    Vt_d = nc.dram_tensor("Vt", [128, 3 * NJ * 4], BF16, kind="ExternalInput")
    Bt_d = nc.dram_tensor("Bt", [128, 3 * NJ], F32, kind="ExternalInput")
    mask_d = nc.dram_tensor("mask1", [1, S_local], BF16, kind="ExternalInput")
    ones_d = nc.dram_tensor("ones3", [1, 3], BF16, kind="ExternalInput")
    id3_d = nc.dram_tensor("id3", [3, 3], BF16, kind="ExternalInput")

    Ncore_d = nc.dram_tensor("Ncore", [3, H2], F32, kind="ExternalOutput")
    stats_d = nc.dram_tensor("stats", [3, 1], F32, kind="ExternalOutput")

    with tile.TileContext(nc) as tc, ExitStack() as ctx:
        const = ctx.enter_context(tc.tile_pool(name="const", bufs=1))
        wpool = ctx.enter_context(tc.tile_pool(name="w", bufs=10))
        thpool = ctx.enter_context(tc.tile_pool(name="th", bufs=6))
        # phase-1 PSUM pools (all 8 banks); closed before the epilogue pools
        # open so the banks can be reused
        ph1 = ExitStack()
        upool = ph1.enter_context(tc.tile_pool(name="u", bufs=3, space="PSUM"))
        epool = ph1.enter_context(tc.tile_pool(name="e", bufs=1, space="PSUM"))

        # ---- sync HWDGE ring: the first weight tiles first (they gate the
        # first matmul), interleaved with sentT chunk-0 per-k 2D transfers ----
        Wt_sb = {}

        def _wdma(t, j):
            tiles = []
            if NK8:
                w8 = wpool.tile([128, NK8 * 128], FP8, tag="w8")
                nc.sync.dma_start(w8[:], W8_d.ap()[t, j])
                tiles.append(w8)
            else:
                tiles.append(None)
            if NKB:
                wb = wpool.tile([128, NKB * 128], BF16, tag="wb")
                nc.sync.dma_start(wb[:], Wb_d.ap()[t, j])
                tiles.append(wb)
            else:
                tiles.append(None)
            Wt_sb[(t, j)] = tiles

        if NK8:
            sT8_sb = const.tile([128, NK8 * S_local], FP8, tag="sT8")
        if NKB:
            sTb_sb = const.tile([128, NKB * S_local], BF16, tag="sTb")

        def _sdma(ring, k, c, n):
            # one [128, n] transfer of sentT k-tile k (fp8 or bf16 partition)
            if k < NK8:
                ring.dma_start(
                    sT8_sb[:, k * S_local + c: k * S_local + c + n],
                    sT8_d.ap()[k * 128:(k + 1) * 128, c:c + n])
            else:
                kb = k - NK8
                ring.dma_start(
                    sTb_sb[:, kb * S_local + c: kb * S_local + c + n],
                    sTb_d.ap()[kb * 128:(kb + 1) * 128, c:c + n])

        c0, n0 = SC[0]
        if NK8:
            # split the gating first W tile per k-pair so the first DoubleRow
            # matmul (which needs only kp=0's 32KB + sentT k0/k1) starts early
            w8_00 = wpool.tile([128, NK8 * 128], FP8, tag="w8")
            for kp in range(pairs):
                # sync ring streams in exact consumption order of (0,0)'s
                # chunk-0 matmuls; the chunk-1 halves ride the idle vector
                # ring so they don't queue ahead of the (0,1..3) weights
                nc.sync.dma_start(w8_00[:, kp * 256:(kp + 1) * 256],
                                  W8_d.ap()[0, 0, :, kp * 256:(kp + 1) * 256])
                for (c, n) in SC:
                    _sdma(nc.sync, 2 * kp, c, n)
                    _sdma(nc.sync, 2 * kp + 1, c, n)
            wb_00 = None
            if NKB:
                wb_00 = wpool.tile([128, NKB * 128], BF16, tag="wb")
                nc.sync.dma_start(wb_00[:], Wb_d.ap()[0, 0])
                for k in range(NK8, KT):
                    _sdma(nc.sync, k, c0, n0)
                for (c, n) in SC[1:]:
                    for k in range(NK8, KT):
                        _sdma(nc.gpsimd, k, c, n)
            Wt_sb[(0, 0)] = [w8_00, wb_00]
            _wdma(0, 1)
            _wdma(0, 2)
            _wdma(0, 3)
        else:
            _wdma(0, 0)
            for k in range(KT // 2):
                _sdma(nc.sync, k, c0, n0)
            _wdma(0, 1)
            for k in range(KT // 2, KT):
                _sdma(nc.sync, k, c0, n0)
            for (c, n) in SC[1:]:
                for k in range(KT):
                    _sdma(nc.gpsimd, k, c, n)
            _wdma(0, 2)
            _wdma(0, 3)

        # ---- consts on the scalar HWDGE ring (separate FIFO) ----
        Vt_sb = const.tile([128, 3 * NJ * 4], BF16, tag="vt")
        Bt_sb = const.tile([128, 3 * NJ], F32, tag="bt")
        mask_sb = const.tile([1, S_local], BF16, tag="mask")
        ones_sb = const.tile([1, 3], BF16, tag="ones")
        id3_sb = const.tile([3, 3], BF16, tag="id3")
        nc.scalar.dma_start(Bt_sb[:], Bt_d.ap()[:])
        nc.scalar.dma_start(Vt_sb[:], Vt_d.ap()[:])
        nc.scalar.dma_start(mask_sb[:], mask_d.ap()[:])
        nc.scalar.dma_start(ones_sb[:], ones_d.ap()[:])
        nc.scalar.dma_start(id3_sb[:], id3_d.ap()[:])

        # ---- the big numerator operand rides the SWDGE ring since it isn't
        # needed until the epilogue ----
        sent_sb = const.tile([128, ST * H2], BF16, tag="sent")
        nc.gpsimd.dma_start(sent_sb[:].rearrange("p (k h) -> p k h", k=ST),
                            sent_d.ap().rearrange("(k p) h -> p k h", p=128))

        # ---- score accumulator [3, S_local]: head t on partition t ----
        e3_ps = epool.tile([3, S_local], F32, tag="e")

        # ---- three heads: u -> tanh -> scores ----
        pend = None  # tanh tile of the previous (t, j), awaiting score matmuls
        for t in range(3):
            for j in range(NJ):
                w8, wb = Wt_sb.pop((t, j), (None, None))
                if NK8 and w8 is None:
                    w8 = wpool.tile([128, NK8 * 128], FP8, tag="w8")
                    nc.sync.dma_start(w8[:], W8_d.ap()[t, j])
                if NKB and wb is None:
                    wb = wpool.tile([128, NKB * 128], BF16, tag="wb")
                    nc.sync.dma_start(wb[:], Wb_d.ap()[t, j])
                u_ps = upool.tile([128, S_local], F32, tag="u")
                if NK8:
                    w8v = w8[:].rearrange("p (kp i m) -> p kp i m",
                                          kp=pairs, i=2)
                    s8v = sT8_sb[:].rearrange("p (k s) -> p k s", k=NK8)
                for kp in range(pairs):
                    for (c, n) in SC:
                        nc.tensor.matmul(
                            u_ps[:, c:c + n],
                            w8v[:, kp],
                            s8v[:, 2 * kp:2 * kp + 2, c:c + n],
                            start=(kp == 0), stop=(kp == pairs - 1 and NKB == 0),
                            perf_mode=DR)
                for kb in range(NKB):
                    for (c, n) in SC:
                        nc.tensor.matmul(
                            u_ps[:, c:c + n],
                            wb[:, kb * 128:(kb + 1) * 128],
                            sTb_sb[:, kb * S_local + c: kb * S_local + c + n],
                            start=(kb == 0 and pairs == 0),
                            stop=(kb == NKB - 1))
                if pend is not None:
                    pth, pt, pj = pend
                    for (c, n) in SC:
                        nc.tensor.matmul(
                            e3_ps[0:3, c:c + n],
                            Vt_sb[:, 4 * (pj * 3 + pt): 4 * (pj * 3 + pt) + 3],
                            pth[:, c:c + n],
                            start=False, stop=(pt == 2 and pj == NJ - 1))
                th = thpool.tile([128, S_local], BF16, tag="th")
                nc.scalar.activation(th[:], u_ps[:], TANH, scale=1.0 / 16.0,
                                     bias=Bt_sb[:, j * 3 + t: j * 3 + t + 1])
                pend = (th, t, j)
                if t == 0 and j == 0:
                    # additive key mask enters the score accumulator via a
                    # K=1 ones-matmul; emitted here (after the first u-group)
                    # so it doesn't head the PE queue at startup, but still
                    # precedes every score matmul
                    for (c, n) in SC:
                        nc.tensor.matmul(e3_ps[0:3, c:c + n], ones_sb[:],
                                         mask_sb[0:1, c:c + n],
                                         start=True, stop=False)
        # preload the Exp activation table while the PE finishes the last
        # score matmuls (the table swap costs ~1.7us on the ScalarE and would
        # otherwise land on the serial epilogue path)
        expwarm = const.tile([1, 3], F32, tag="expwarm")
        nc.scalar.activation(expwarm[:], ones_sb[:], EXP)

        pth, pt, pj = pend
        for (c, n) in SC:
            nc.tensor.matmul(e3_ps[0:3, c:c + n],
                             Vt_sb[:, 4 * (pj * 3 + pt): 4 * (pj * 3 + pt) + 3],
                             pth[:, c:c + n], start=False,
                             stop=True)

        # ---- no-max softmax: exp straight off PSUM (|e| <= ~36 so exp fits
        # fp32 with room), accumulating Z along the way ----
        e3x_sb = const.tile([3, S_local], BF16, tag="e3x")
        Z3 = const.tile([3, 1], F32, tag="z3")
        SCE = [(c, min(256, S_local - c)) for c in range(0, S_local, 256)]
        zpart = const.tile([3, len(SCE)], F32, tag="zpart")
        for ci, (c, n) in enumerate(SCE):
            nc.scalar.activation(e3x_sb[0:3, c:c + n], e3_ps[0:3, c:c + n], EXP,
                                 accum_out=zpart[:, ci:ci + 1])
        if len(SCE) > 1:
            nc.vector.reduce_sum(Z3[:, 0:1], zpart[:], axis=mybir.AxisListType.X)
        else:
            nc.vector.tensor_copy(Z3[:, 0:1], zpart[:, 0:1])
        stats_sb = const.tile([3, 1], F32, tag="stats")
        nc.vector.tensor_copy(stats_sb[:, 0:1], Z3[:, 0:1])
        nc.scalar.dma_start(stats_d.ap()[:], stats_sb[:])

        ph1.close()  # free u/e PSUM banks for the epilogue pools

        # ---- fused epilogue: per s-tile, transpose exp-scores to [s, 3]
        # and immediately accumulate both H2 halves of the numerator
        # N[t, :] = sum_s exp_scores[t, s] * sent[s, :] ----
        trpool = ctx.enter_context(tc.tile_pool(name="tr", bufs=2, space="PSUM"))
        npool = ctx.enter_context(tc.tile_pool(name="n", bufs=2, space="PSUM"))
        eT_sb = const.tile([128, 4 * ST], BF16, tag="eT")
        n_ps = []
        for _hi in range(H2 // 512):
            n_ps_hi = npool.tile([3, 512], F32, tag="n")
            n_ps.append(n_ps_hi)
        for k in range(ST):
            tr_ps = trpool.tile([128, 3], BF16, tag="tr")
            nc.tensor.transpose(tr_ps[:], e3x_sb[0:3, k * 128:(k + 1) * 128],
                                id3_sb[:])
            nc.vector.tensor_copy(eT_sb[:, 4 * k:4 * k + 3], tr_ps[:])
            for hi, hc in enumerate(range(0, H2, 512)):
                nc.tensor.matmul(n_ps[hi][0:3, :],
                                 eT_sb[:, 4 * k:4 * k + 3],
                                 sent_sb[:, k * H2 + hc: k * H2 + hc + 512],
                                 start=(k == 0), stop=(k == ST - 1))
        n_sb = const.tile([3, H2], F32, tag="nsb")
        for hi, hc in enumerate(range(0, H2, 512)):
            nc.vector.tensor_copy(n_sb[:, hc:hc + 512], n_ps[hi][:])
            nc.sync.dma_start(Ncore_d.ap()[:, hc:hc + 512], n_sb[:, hc:hc + 512])

    nc.compile()
    return nc


def kernel(**inputs):
    global LAST_RESULTS
    import ml_dtypes
    from concourse import bass_utils

    E4 = ml_dtypes.float8_e4m3
    BF = ml_dtypes.bfloat16

    sentence = np.ascontiguousarray(
        np.asarray(inputs["sentence"], dtype=np.float32)[0])      # [S, H2]
    length = int(np.asarray(inputs["length"]).reshape(-1)[0])
    if length <= 0:
        return np.zeros((1, H2), dtype=np.float32)
    length = min(length, S)

    ctxs = [inputs["pos_embedding"], inputs["cardinal_phrase_embedding"],
            inputs["headline_embedding"]]
    tags = ["p", "c", "h"]

    # host-side prep: fold ctx projection + b_sent into a single bias [3, A]
    bias_all = np.empty((3, A), dtype=np.float32)
    W_all = np.empty((3, H2, A), dtype=np.float32)
    v_all = np.empty((3, A), dtype=np.float32)
    for i, tg in enumerate(tags):
        ctx = np.asarray(ctxs[i], dtype=np.float32)[0]            # [E]
        bias_all[i] = (np.asarray(inputs[f"b_sent_{tg}"], dtype=np.float32)
                       + ctx @ np.asarray(inputs[f"W_ctx_{tg}"], dtype=np.float32)
                       + np.asarray(inputs[f"b_ctx_{tg}"], dtype=np.float32))
        W_all[i] = np.asarray(inputs[f"W_sent_{tg}"], dtype=np.float32)
        v_all[i] = np.asarray(inputs[f"v_{tg}"], dtype=np.float32)

    pairs = U_FP8_PAIRS
    NK8 = 2 * pairs
    KT = H2 // 128
    NKB = KT - NK8
    S_local = max(128, -(-length // (NCORES * 128)) * 128)        # ceil, 128-aligned
    nc = _cache.get((S_local, pairs))
    if nc is None:
        nc = _build(S_local, pairs)
        _cache[(S_local, pairs)] = nc

    NJ = A // 128
    # W tiles, k-tile major per (t, j):  [3, NJ, 128, KT, 128] with the
    # partition dim holding the low 7 bits of the contraction index
    Wt = (W_all.reshape(3, KT, 128, NJ, 128)
               .transpose(0, 3, 2, 1, 4))                         # [3,NJ,128,KT,128]
    if NK8:
        # fp8 pairs carry W*8 (and x*2) for a uniform x16 PSUM scale
        W8 = np.ascontiguousarray(
            np.clip(Wt[:, :, :, :NK8] * 8.0, -240, 240)).astype(E4)
        W8 = np.ascontiguousarray(W8.reshape(3, NJ, 128, NK8 * 128))
    if NKB:
        Wb = np.ascontiguousarray(Wt[:, :, :, NK8:] * 16.0).astype(BF)
        Wb = np.ascontiguousarray(Wb.reshape(3, NJ, 128, NKB * 128))

    # [128, (j t) * 3]: head t's v-tile in column t of its [128, 3] block
    vt_cols = v_all.T.reshape(NJ, 128, 3).transpose(1, 0, 2)      # [128, NJ, 3]
    Vt = np.zeros((128, NJ, 3, 4), dtype=np.float32)
    for t in range(3):
        Vt[:, :, t, t] = vt_cols[:, :, t]
    Vt = np.ascontiguousarray(Vt.reshape(128, 3 * NJ * 4)).astype(BF)
    Bt = np.ascontiguousarray(
        bias_all.T.reshape(NJ, 128, 3).transpose(1, 0, 2).reshape(128, 3 * NJ))
    id3 = np.eye(3, dtype=np.float32).astype(BF)
    ones3 = np.ones((1, 3), dtype=np.float32).astype(BF)

    in_maps = []
    for c in range(NCORES):
        s0 = c * S_local
        sl = sentence[s0:s0 + S_local]
        if sl.shape[0] < S_local:                                  # pad tail core
            sl = np.concatenate(
                [sl, np.zeros((S_local - sl.shape[0], H2), np.float32)], axis=0)
        mask1 = np.where((s0 + np.arange(S_local))[None, :] < length,
                         0.0, NEG).astype(np.float32).astype(BF)
        slT = sl.T                                                 # [H2, S_local]
        im = dict(Vt=Vt, Bt=Bt, mask1=mask1, ones3=ones3, id3=id3,
                  sent=np.ascontiguousarray(sl).astype(BF))
        if NK8:
            im["sT8"] = np.ascontiguousarray(
                np.clip(slT[:NK8 * 128] * 2.0, -240, 240)).astype(E4)
            im["W8"] = W8
        if NKB:
            im["sTb"] = np.ascontiguousarray(slT[NK8 * 128:]).astype(BF)
            im["Wb"] = Wb
        in_maps.append(im)

    res = bass_utils.run_bass_kernel_spmd(nc, in_maps,
                                          core_ids=list(range(NCORES)))
    LAST_RESULTS = res

    # ---- exact cross-core softmax combine: plain sums (no max shift) ----
    Z = np.zeros(3, dtype=np.float64)
    N = np.zeros((3, H2), dtype=np.float64)
    for c in range(NCORES):
        Z += res.results[c]["stats"][:, 0].astype(np.float64)
        N += res.results[c]["Ncore"].astype(np.float64)
    out = (N / Z[:, None]).mean(axis=0)
    return out[None, :].astype(np.float32)
